# revision 24
# baseline (speedup 1.0000x reference)
"""Causal self-attention (GPT-style block) on 8 Trainium2 NeuronCores.

Problem: x[4, 2048, 768], w_attn[2304, 768], b_attn[2304], w_proj[768, 768],
b_proj[768]; 12 heads of size 64; causal softmax attention; output [4, 2048, 768].

Sharding: batch x heads. core = 2*b + g handles batch b (of 4) and the 6 heads
g*6..g*6+5 (tensor parallel over heads). Each core:
  1. QKV projection per 512-token chunk, producing Q^T/K^T in [r, t] layout and
     V in [t, r] layout (with a fused ones column for softmax denominators).
     Biases are folded into the PSUM->SBUF evacuation (tensor_scalar add with a
     per-partition bias column for Q/K; tensor_tensor add with a broadcast bias
     tile for V) so no PE cycles are spent on bias.
  2. Flash-style causal attention per head pair (kb): S^T tiles [128 kv, 512 q]
     on PE, exp on ACT (scale=1/8), triangular mask on the diagonal 128x128
     block via DVE, O^T accumulation on PE with the V-ones column yielding the
     softmax denominator for free. Normalization per pair: two reciprocal rows
     -> one rank-1 matmul (E2^T @ rec2) broadcasts both across the pair's 128
     partitions -> two DVE multiplies.
  3. c_proj with its 384 local channels -> partial y[2048, 768].
  4. Host reassembles: partial sums of core pairs plus b_proj.

QKV projection of chunk J+1 is emitted after attention of chunk J with
per-chunk activation tiles, so the list scheduler uses projection matmuls to
fill PE gaps in the ACT-bound attention inner loop. All matmuls run as float32r
(TF32-like, 1 cycle/row at N>=256) with fp32 PSUM accumulation.

build_bass(loop_reps=R) wraps the whole per-iteration body in a hardware For_i
loop: used by test.py to time the steady-state body as (t_R - t_1)/(R - 1),
amplifying device time far above host/dispatch jitter.
"""
import os

import numpy as np

os.environ.setdefault("JAX_COMPILATION_CACHE_DIR", "/tmp/jaxcache")
os.environ.setdefault("JAX_PERSISTENT_CACHE_MIN_COMPILE_TIME_SECS", "0")
os.environ.setdefault("JAX_PERSISTENT_CACHE_MIN_ENTRY_SIZE_BYTES", "0")

import concourse.bass as bass
import concourse.bacc as bacc
import concourse.tile as tile
from concourse import mybir
from concourse.bass_utils import run_bass_kernel_spmd

B, T, C, H = 4, 2048, 768, 12
HS = 64          # head size
HL = 6           # heads per core
CL = HL * HS     # 384 local channels per core
NQ = 512         # q block width
NCH = T // NQ    # 4 chunks
NCORES = 8
F32 = mybir.dt.float32
F32R = mybir.dt.float32r
EXP = mybir.ActivationFunctionType.Exp
ADD = mybir.AluOpType.add


def build_bass(loop_reps=None):
    nc = bacc.Bacc(num_devices=NCORES)
    xT = nc.declare_dram_parameter("xT", [C, T], F32, isOutput=False)
    wqkT = nc.declare_dram_parameter("wqkT", [C, 2 * CL], F32, isOutput=False)
    wvT = nc.declare_dram_parameter("wvT", [C, CL], F32, isOutput=False)
    wpT = nc.declare_dram_parameter("wpT", [CL, C], F32, isOutput=False)
    bqkc = nc.declare_dram_parameter("bqkc", [128, 6], F32, isOutput=False)
    bv = nc.declare_dram_parameter("bv", [1, CL], F32, isOutput=False)
    tri = nc.declare_dram_parameter("tri", [128, 128], F32, isOutput=False)
    ones = nc.declare_dram_parameter("ones", [128, 128], F32, isOutput=False)
    e2 = nc.declare_dram_parameter("e2", [65, 128], F32, isOutput=False)
    y_out = nc.declare_dram_parameter("y_out", [T, C], F32, isOutput=True)

    with tile.TileContext(nc) as tc:
        with (
            tc.tile_pool(name="const", bufs=1) as constp,
            tc.tile_pool(name="wpool", bufs=1) as wpool,
            tc.tile_pool(name="qkv", bufs=1) as qkvp,
            tc.tile_pool(name="xch", bufs=2) as xchp,
            tc.tile_pool(name="ptp", bufs=4) as ptp,
            tc.tile_pool(name="otsb", bufs=3) as otsbp,
            tc.tile_pool(name="small", bufs=2) as smallp,
            tc.tile_pool(name="yev", bufs=2) as yevp,
            tc.tile_pool(name="ps_s", bufs=2, space="PSUM") as pss,
            tc.tile_pool(name="ps_qkv", bufs=1, space="PSUM") as psqkv,
            tc.tile_pool(name="ps_ot", bufs=2, space="PSUM") as psot,
            tc.tile_pool(name="ps_y", bufs=1, space="PSUM") as psy,
        ):
            xTr = xT[:, :].bitcast(F32R).rearrange("(cb p) t -> p cb t", p=128)

            # first x chunk DMA goes out before the weight bulk so the PE can
            # start QKV as early as possible
            xc0 = xchp.tile([128, 6, NQ], F32R, tag="xc")
            for cb in range(6):
                (nc.sync if cb % 2 == 0 else nc.gpsimd).dma_start(
                    out=xc0[:, cb, :], in_=xTr[:, cb, 0:NQ])

            wqk_sb = []
            for cb in range(6):
                wt = wpool.tile([128, 2 * CL], F32R, tag=f"wqk{cb}")
                (nc.sync if cb % 2 == 0 else nc.gpsimd).dma_start(
                    out=wt, in_=wqkT[cb * 128:(cb + 1) * 128, :].bitcast(F32R))
                wqk_sb.append(wt)
            wv_sb = []
            for cb in range(6):
                wt = wpool.tile([128, CL], F32R, tag=f"wv{cb}")
                (nc.sync if cb % 2 == 0 else nc.gpsimd).dma_start(
                    out=wt, in_=wvT[cb * 128:(cb + 1) * 128, :].bitcast(F32R))
                wv_sb.append(wt)
            tri_sb = constp.tile([128, 128], F32, tag="tri")
            nc.sync.dma_start(out=tri_sb, in_=tri[:, :])
            bqk_sb = constp.tile([128, 6], F32, tag="bqkc")
            nc.gpsimd.dma_start(out=bqk_sb, in_=bqkc[:, :])
            bv_sb = constp.tile([1, CL], F32R, tag="bv")
            nc.sync.dma_start(out=bv_sb, in_=bv[:, :].bitcast(F32R))
            ones_sb = constp.tile([1, 128], F32R, tag="ones")
            nc.gpsimd.dma_start(out=ones_sb, in_=ones[0:1, :].bitcast(F32R))
            e2w_sb = constp.tile([65, 128], F32R, tag="e2w")
            nc.sync.dma_start(out=e2w_sb, in_=e2[:, :].bitcast(F32R))
            # persistent reciprocal-rows tile: rows 1..63 stay zero so the
            # 65-row rank-1 broadcast matmul only sees rows 0 and 64
            rec2 = constp.tile([65, NQ], F32R, tag="rec2")
            nc.vector.memset(rec2[1:HS, :], 0.0)
            wp_sb = []
            for cb in range(3):
                wt = wpool.tile([128, C], F32R, tag=f"wp{cb}")
                (nc.sync if cb % 2 == 0 else nc.gpsimd).dma_start(
                    out=wt, in_=wpT[cb * 128:(cb + 1) * 128, :].bitcast(F32R))
                wp_sb.append(wt)

            # broadcast V bias to all 128 partitions once: bv128 = ones^T @ bv
            bvps = psy.tile([128, CL], F32, tag="yps")
            nc.tensor.matmul(bvps, lhsT=ones_sb, rhs=bv_sb, start=True,
                             stop=True)
            bv128 = constp.tile([128, CL], F32, tag="bv128")
            nc.vector.tensor_copy(bv128, bvps)

            ctx = dict(nc=nc, xTr=xTr, wqk_sb=wqk_sb, wv_sb=wv_sb,
                       wp_sb=wp_sb, bqk_sb=bqk_sb, bv128=bv128, tri_sb=tri_sb,
                       y_out=y_out, xc0=xc0, ones=ones,
                       e2w_sb=e2w_sb, rec2=rec2,
                       xchp=xchp, qkvp=qkvp,
                       ptp=ptp, otsbp=otsbp, smallp=smallp, yevp=yevp,
                       pss=pss, psqkv=psqkv, psot=psot, psy=psy)

            if loop_reps is None or loop_reps <= 1:
                body(ctx, first=True)
            else:
                with tc.For_i(0, loop_reps, 1,
                              hint_engines=(mybir.EngineType.PE,
                                            mybir.EngineType.Activation,
                                            mybir.EngineType.DVE)):
                    body(ctx, first=False)
    nc.finalize()
    return nc


def emit_xc_dma(ctx, tcn):
    """DMA one 512-token chunk of x^T into SBUF (issued on SP+Pool)."""
    nc = ctx["nc"]
    xc = ctx["xchp"].tile([128, 6, NQ], F32R, tag="xc")
    for cb in range(6):
        (nc.sync if cb % 2 == 0 else nc.gpsimd).dma_start(
            out=xc[:, cb, :], in_=ctx["xTr"][:, cb, tcn * NQ:(tcn + 1) * NQ])
    return xc


def emit_qkv_q(ctx, tcn, QTc, xc):
    """Q projection (row-blocks 0-2) for chunk tcn. Chunk 0 (prologue, no
    attention running yet) borrows the S-pair pool for double buffering."""
    nc = ctx["nc"]
    wqk_sb, bqk_sb = ctx["wqk_sb"], ctx["bqk_sb"]
    pool, tag = (ctx["pss"], "s") if tcn == 0 else (ctx["psqkv"], "qk")
    for rb in range(3):
        ps = pool.tile([128, NQ], F32, tag=tag)
        for cb in range(6):
            nc.tensor.matmul(
                ps, lhsT=wqk_sb[cb][:, rb * 128:(rb + 1) * 128],
                rhs=xc[:, cb, :], start=(cb == 0), stop=(cb == 5))
        with nc.allow_low_precision(reason="fp32r matmul operand"):
            nc.vector.tensor_scalar_add(QTc[rb][tcn], ps,
                                        bqk_sb[:, rb:rb + 1])


def emit_qkv_kv(ctx, tcn, KTc, Vc, xc):
    """K (row-blocks 3-5) + V projection for chunk tcn."""
    nc = ctx["nc"]
    wqk_sb, wv_sb = ctx["wqk_sb"], ctx["wv_sb"]
    bqk_sb, bv128 = ctx["bqk_sb"], ctx["bv128"]
    pool, tag = (ctx["pss"], "s") if tcn == 0 else (ctx["psqkv"], "qk")
    for rb in range(3, 6):
        ps = pool.tile([128, NQ], F32, tag=tag)
        for cb in range(6):
            nc.tensor.matmul(
                ps, lhsT=wqk_sb[cb][:, rb * 128:(rb + 1) * 128],
                rhs=xc[:, cb, :], start=(cb == 0), stop=(cb == 5))
        with nc.allow_low_precision(reason="fp32r matmul operand"):
            nc.vector.tensor_scalar_add(KTc[rb - 3][tcn], ps,
                                        bqk_sb[:, rb:rb + 1])
    # V: [t, r] layout, 4 t-subblocks; ones column fused for denominators
    Vc = Vc[tcn]
    nc.gpsimd.dma_start(
        out=Vc[:, :, :, HS],
        in_=ctx["ones"][:, 0:4 * HL].bitcast(F32R).rearrange(
            "p (a b) -> p a b", b=HL))
    for tb in range(4):
        psv = pool.tile([128, CL], F32, tag=tag)
        for cb in range(6):
            nc.tensor.matmul(
                psv, lhsT=xc[:, cb, tb * 128:(tb + 1) * 128],
                rhs=wv_sb[cb], start=(cb == 0), stop=(cb == 5))
        with nc.allow_low_precision(reason="fp32r matmul operand"):
            nc.vector.tensor_tensor(
                out=Vc[:, tb, :, 0:HS],
                in0=psv.rearrange("p (h d) -> p h d", d=HS),
                in1=bv128.rearrange("p (h d) -> p h d", d=HS), op=ADD)


def body(ctx, first):
    nc = ctx["nc"]
    qkvp = ctx["qkvp"]
    wp_sb, tri_sb = ctx["wp_sb"], ctx["tri_sb"]
    y_out = ctx["y_out"]
    ptp, otsbp, smallp, yevp = (ctx["ptp"], ctx["otsbp"], ctx["smallp"],
                                ctx["yevp"])
    pss, psot, psy = ctx["pss"], ctx["psot"], ctx["psy"]
    e2w_sb, rec2 = ctx["e2w_sb"], ctx["rec2"]

    # per-chunk activation tiles: fine-grained deps let chunk J+1's QKV
    # projection overlap chunk J's attention
    QTc = [[qkvp.tile([128, NQ], F32R, tag=f"qt{rb}_{j}", name=f"qt{rb}_{j}")
            for j in range(NCH)] for rb in range(3)]
    KTc = [[qkvp.tile([128, NQ], F32R, tag=f"kt{rb}_{j}", name=f"kt{rb}_{j}")
            for j in range(NCH)] for rb in range(3)]
    Vc = [qkvp.tile([128, 4, HL, HS + 1], F32R, tag=f"v{j}", name=f"v{j}")
          for j in range(NCH)]

    xc = ctx["xc0"] if first else emit_xc_dma(ctx, 0)
    emit_qkv_q(ctx, 0, QTc, xc)
    emit_qkv_kv(ctx, 0, KTc, Vc, xc)

    for J in range(NCH):
        if J + 1 < NCH:
            # issue next chunk's x DMA early (SP/Pool are free)
            xcn = emit_xc_dma(ctx, J + 1)
        ots = [otsbp.tile([128, NQ], F32R, tag=f"ots{kb}", name=f"ots{kb}")
               for kb in range(3)]
        for kb in range(3):
            ot = [psot.tile([HS + 1, NQ], F32, tag="ot", name=f"ot{_h}")
                  for _h in range(2)]
            # software pipeline: issue S(t)/exp(t) for both heads, then the
            # deferred O(t-1) pair, so the PE works ahead of ACT's exp.
            pend = []
            for t in range(J * 4 + 4):
                tc_i, tb = divmod(t, 4)
                diag = t - J * 4             # >= 0 on diagonal tiles
                W = NQ if diag < 0 else NQ - 128 * diag
                new = []
                # both heads' S^T tiles go into one 2-bank PSUM tile so a
                # single ACT instruction computes exp for the pair
                sps = pss.tile([128, 2, NQ], F32, tag="s")
                pt = ptp.tile([128, 2, NQ], F32R, tag="pt")
                for hh in range(2):
                    po = hh * HS
                    kt = KTc[kb][tc_i][po:po + HS, tb * 128:(tb + 1) * 128]
                    nc.tensor.matmul(
                        sps[:, hh, 0:W], lhsT=kt,
                        rhs=(QTc[kb][J][po:po + HS, :] if diag < 0 else
                             QTc[kb][J][po:po + HS, 128 * diag:NQ]),
                        start=True, stop=True)
                nc.scalar.activation(pt[:, :, 0:W], sps[:, :, 0:W], EXP,
                                     scale=0.125)
                if diag >= 0:
                    nc.vector.tensor_mul(pt[:, 0, 0:128], pt[:, 0, 0:128],
                                         tri_sb)
                    nc.vector.tensor_mul(pt[:, 1, 0:128], pt[:, 1, 0:128],
                                         tri_sb)
                for hh in range(2):
                    dst = ot[hh] if diag < 0 else ot[hh][:, 128 * diag:NQ]
                    new.append(dict(out=dst, lhsT=Vc[tc_i][:, tb, 2 * kb + hh, :],
                                    rhs=pt[:, hh, 0:W], start=(t == 0),
                                    stop=False))
                for o in pend:
                    nc.tensor.matmul(o.pop("out"), **o)
                pend = new
            for o in pend:
                o["stop"] = True
                nc.tensor.matmul(o.pop("out"), **o)
            # normalize the pair: reciprocal rows -> rank-1 broadcast -> mul
            with nc.allow_low_precision(reason="fp32r matmul operand"):
                nc.vector.reciprocal(rec2[0:1, :], ot[0][HS:HS + 1, :])
                nc.vector.reciprocal(rec2[HS:HS + 1, :], ot[1][HS:HS + 1, :])
            bc = psy.tile([128, NQ], F32, tag="yps")
            nc.tensor.matmul(bc, lhsT=e2w_sb, rhs=rec2, start=True, stop=True)
            bcs = smallp.tile([128, NQ], F32, tag="bcs")
            nc.vector.tensor_copy(bcs, bc)
            nc.vector.tensor_mul(ots[kb][0:HS, :], ot[0][0:HS, :],
                                 bcs[0:HS, :])
            nc.vector.tensor_mul(ots[kb][HS:128, :], ot[1][0:HS, :],
                                 bcs[HS:128, :])
        # c_proj for this q-block
        for i in range(4):
            ti = J * 4 + i
            yt = yevp.tile([128, C], F32, tag="yt")
            for half in range(2):
                yps = psy.tile([128, CL], F32, tag="yps")
                for cb in range(3):
                    nc.tensor.matmul(
                        yps, lhsT=ots[cb][:, i * 128:(i + 1) * 128],
                        rhs=wp_sb[cb][:, half * CL:(half + 1) * CL],
                        start=(cb == 0), stop=(cb == 2))
                nc.vector.tensor_copy(yt[:, half * CL:(half + 1) * CL], yps)
            nc.sync.dma_start(out=y_out[ti * 128:(ti + 1) * 128, :], in_=yt)
        # project the next chunk, emitted last (= lowest priority): the
        # scheduler pulls these matmuls into PE stalls of the ACT-bound
        # attention loops above
        if J + 1 < NCH:
            emit_qkv_q(ctx, J + 1, QTc, xcn)
            emit_qkv_kv(ctx, J + 1, KTc, Vc, xcn)


def make_in_maps(x, w_attn, b_attn, w_proj):
    x = np.asarray(x, dtype=np.float32)
    w_attn = np.asarray(w_attn, dtype=np.float32)
    b_attn = np.asarray(b_attn, dtype=np.float32)
    w_proj = np.asarray(w_proj, dtype=np.float32)
    # valid iff kv <= q with kv on partitions (rows), q on free dim (cols)
    tri = np.triu(np.ones((128, 128), dtype=np.float32))
    e2 = np.zeros((65, 128), dtype=np.float32)
    e2[0, 0:HS] = 1.0
    e2[HS, HS:128] = 1.0
    in_maps = []
    for core in range(NCORES):
        b, g = divmod(core, 2)
        sl = slice(g * CL, (g + 1) * CL)
        wq, wk, wv = (w_attn[i * C:(i + 1) * C][sl] for i in range(3))
        bq, bk, bv_ = (b_attn[i * C:(i + 1) * C][sl] for i in range(3))
        bqkc = np.concatenate([bq, bk]).reshape(6, 128).T.copy()
        in_maps.append({
            "xT": np.ascontiguousarray(x[b].T),
            "wqkT": np.ascontiguousarray(np.concatenate([wq, wk], 0).T),
            "wvT": np.ascontiguousarray(wv.T),
            "wpT": np.ascontiguousarray(w_proj[:, sl].T),
            "bqkc": bqkc,
            "bv": bv_[None, :].copy(),
            "tri": tri,
            "ones": np.ones((128, 128), dtype=np.float32),
            "e2": e2,
        })
    return in_maps


def assemble(results, b_proj):
    out = np.empty((B, T, C), dtype=np.float32)
    for b in range(B):
        out[b] = results[2 * b]["y_out"] + results[2 * b + 1]["y_out"]
    out += np.asarray(b_proj, dtype=np.float32)[None, None, :]
    return out


_CACHE = {}


def _get_nc():
    if "nc" not in _CACHE:
        _CACHE["nc"] = build_bass()
    return _CACHE["nc"]


def kernel(x, w_attn, b_attn, w_proj, b_proj):
    in_maps = make_in_maps(x, w_attn, b_attn, w_proj)
    res = run_bass_kernel_spmd(_get_nc(), in_maps, list(range(NCORES)))
    return assemble(res.results, b_proj)


# revision 28
# speedup vs baseline: 1.0260x; 1.0260x over previous
"""Causal self-attention (GPT-style block) on 8 Trainium2 NeuronCores.

Problem: x[4, 2048, 768], w_attn[2304, 768], b_attn[2304], w_proj[768, 768],
b_proj[768]; 12 heads of size 64; causal softmax attention; output [4, 2048, 768].

Sharding: batch x heads. core = 2*b + g handles batch b (of 4) and the 6 heads
g*6..g*6+5 (tensor parallel over heads). Each core:
  1. QKV projection per 512-token chunk, producing Q^T/K^T in [r, t] layout and
     V in [t, r] layout (with a fused ones column for softmax denominators).
     Biases are folded into the PSUM->SBUF evacuation (tensor_scalar add with a
     per-partition bias column for Q/K; tensor_tensor add with a broadcast bias
     tile for V) so no PE cycles are spent on bias.
  2. Flash-style causal attention per head pair (kb): S^T tiles [128 kv, 512 q]
     on PE, exp on ACT (scale=1/8), triangular mask on the diagonal 128x128
     block via DVE, O^T accumulation on PE with the V-ones column yielding the
     softmax denominator for free. Normalization per pair: two reciprocal
     rows -> two accumulating rank-1 matmuls broadcast them across the pair's
     128 partitions -> two DVE multiplies.
  3. c_proj with its 384 local channels -> partial y[2048, 768].
  4. Host reassembles: partial sums of core pairs plus b_proj.

QKV projection of chunk J+1 is emitted after attention of chunk J with
per-chunk activation tiles, so the list scheduler uses projection matmuls to
fill PE gaps in the ACT-bound attention inner loop. Exp runs one ACT
instruction per head pair over a 2-bank PSUM S tile. All matmuls run as
float32r (TF32-like, 1 cycle/row at N>=256) with fp32 PSUM accumulation.

build_bass(loop_reps=R) wraps the whole per-iteration body in a hardware For_i
loop: used by test.py to time the steady-state body as (t_R - t_1)/(R - 1),
amplifying device time far above host/dispatch jitter.
"""
import os

import numpy as np

os.environ.setdefault("JAX_COMPILATION_CACHE_DIR", "/tmp/jaxcache")
os.environ.setdefault("JAX_PERSISTENT_CACHE_MIN_COMPILE_TIME_SECS", "0")
os.environ.setdefault("JAX_PERSISTENT_CACHE_MIN_ENTRY_SIZE_BYTES", "0")

import concourse.bass as bass
import concourse.bacc as bacc
import concourse.tile as tile
from concourse import mybir
from concourse.bass_utils import run_bass_kernel_spmd

B, T, C, H = 4, 2048, 768, 12
HS = 64          # head size
HL = 6           # heads per core
CL = HL * HS     # 384 local channels per core
NQ = 512         # q block width
NCH = T // NQ    # 4 chunks
NCORES = 8
F32 = mybir.dt.float32
F32R = mybir.dt.float32r
EXP = mybir.ActivationFunctionType.Exp
ADD = mybir.AluOpType.add


def build_bass(loop_reps=None):
    nc = bacc.Bacc(num_devices=NCORES)
    xT = nc.declare_dram_parameter("xT", [C, T], F32, isOutput=False)
    wqkT = nc.declare_dram_parameter("wqkT", [C, 2 * CL], F32, isOutput=False)
    wvT = nc.declare_dram_parameter("wvT", [C, CL], F32, isOutput=False)
    wpT = nc.declare_dram_parameter("wpT", [CL, C], F32, isOutput=False)
    bqkc = nc.declare_dram_parameter("bqkc", [128, 6], F32, isOutput=False)
    bv = nc.declare_dram_parameter("bv", [1, CL], F32, isOutput=False)
    tri = nc.declare_dram_parameter("tri", [128, 128], F32, isOutput=False)
    ones = nc.declare_dram_parameter("ones", [128, 128], F32, isOutput=False)
    e2 = nc.declare_dram_parameter("e2", [2, 128], F32, isOutput=False)
    y_out = nc.declare_dram_parameter("y_out", [T, C], F32, isOutput=True)

    with tile.TileContext(nc) as tc:
        with (
            tc.tile_pool(name="const", bufs=1) as constp,
            tc.tile_pool(name="wpool", bufs=1) as wpool,
            tc.tile_pool(name="qkv", bufs=1) as qkvp,
            tc.tile_pool(name="xch", bufs=2) as xchp,
            tc.tile_pool(name="ptp", bufs=4) as ptp,
            tc.tile_pool(name="otsb", bufs=3) as otsbp,
            tc.tile_pool(name="small", bufs=2) as smallp,
            tc.tile_pool(name="yev", bufs=2) as yevp,
            tc.tile_pool(name="ps_s", bufs=2, space="PSUM") as pss,
            tc.tile_pool(name="ps_qkv", bufs=1, space="PSUM") as psqkv,
            tc.tile_pool(name="ps_ot", bufs=2, space="PSUM") as psot,
            tc.tile_pool(name="ps_y", bufs=1, space="PSUM") as psy,
        ):
            xTr = xT[:, :].bitcast(F32R).rearrange("(cb p) t -> p cb t", p=128)

            # first x chunk DMA goes out before the weight bulk so the PE can
            # start QKV as early as possible
            xc0 = xchp.tile([128, 6, NQ], F32R, tag="xc")
            for cb in range(6):
                (nc.sync if cb % 2 == 0 else nc.gpsimd).dma_start(
                    out=xc0[:, cb, :], in_=xTr[:, cb, 0:NQ])

            wqk_sb = []
            for cb in range(6):
                wt = wpool.tile([128, 2 * CL], F32R, tag=f"wqk{cb}")
                (nc.sync if cb % 2 == 0 else nc.gpsimd).dma_start(
                    out=wt, in_=wqkT[cb * 128:(cb + 1) * 128, :].bitcast(F32R))
                wqk_sb.append(wt)
            wv_sb = []
            for cb in range(6):
                wt = wpool.tile([128, CL], F32R, tag=f"wv{cb}")
                (nc.sync if cb % 2 == 0 else nc.gpsimd).dma_start(
                    out=wt, in_=wvT[cb * 128:(cb + 1) * 128, :].bitcast(F32R))
                wv_sb.append(wt)
            tri_sb = constp.tile([128, 128], F32, tag="tri")
            nc.sync.dma_start(out=tri_sb, in_=tri[:, :])
            bqk_sb = constp.tile([128, 6], F32, tag="bqkc")
            nc.gpsimd.dma_start(out=bqk_sb, in_=bqkc[:, :])
            bv_sb = constp.tile([1, CL], F32R, tag="bv")
            nc.sync.dma_start(out=bv_sb, in_=bv[:, :].bitcast(F32R))
            ones_sb = constp.tile([1, 128], F32R, tag="ones")
            nc.gpsimd.dma_start(out=ones_sb, in_=ones[0:1, :].bitcast(F32R))
            e2a_sb = constp.tile([1, 128], F32R, tag="e2a")
            nc.sync.dma_start(out=e2a_sb, in_=e2[0:1, :].bitcast(F32R))
            e2b_sb = constp.tile([1, 128], F32R, tag="e2b")
            nc.sync.dma_start(out=e2b_sb, in_=e2[1:2, :].bitcast(F32R))
            wp_sb = []
            for cb in range(3):
                wt = wpool.tile([128, C], F32R, tag=f"wp{cb}")
                (nc.sync if cb % 2 == 0 else nc.gpsimd).dma_start(
                    out=wt, in_=wpT[cb * 128:(cb + 1) * 128, :].bitcast(F32R))
                wp_sb.append(wt)

            # broadcast V bias to all 128 partitions once: bv128 = ones^T @ bv
            bvps = psy.tile([128, CL], F32, tag="yps")
            nc.tensor.matmul(bvps, lhsT=ones_sb, rhs=bv_sb, start=True,
                             stop=True)
            bv128 = constp.tile([128, CL], F32, tag="bv128")
            nc.vector.tensor_copy(bv128, bvps)

            ctx = dict(nc=nc, xTr=xTr, wqk_sb=wqk_sb, wv_sb=wv_sb,
                       wp_sb=wp_sb, bqk_sb=bqk_sb, bv128=bv128, tri_sb=tri_sb,
                       y_out=y_out, xc0=xc0, ones=ones,
                       e2a_sb=e2a_sb, e2b_sb=e2b_sb,
                       xchp=xchp, qkvp=qkvp,
                       ptp=ptp, otsbp=otsbp, smallp=smallp, yevp=yevp,
                       pss=pss, psqkv=psqkv, psot=psot, psy=psy)

            if loop_reps is None or loop_reps <= 1:
                body(ctx, first=True)
            else:
                with tc.For_i(0, loop_reps, 1,
                              hint_engines=(mybir.EngineType.PE,
                                            mybir.EngineType.Activation,
                                            mybir.EngineType.DVE)):
                    body(ctx, first=False)
    nc.finalize()
    return nc


def emit_xc_dma(ctx, tcn):
    """DMA one 512-token chunk of x^T into SBUF (issued on SP+Pool)."""
    nc = ctx["nc"]
    xc = ctx["xchp"].tile([128, 6, NQ], F32R, tag="xc")
    for cb in range(6):
        (nc.sync if cb % 2 == 0 else nc.gpsimd).dma_start(
            out=xc[:, cb, :], in_=ctx["xTr"][:, cb, tcn * NQ:(tcn + 1) * NQ])
    return xc


def emit_qkv_q(ctx, tcn, QTc, xc):
    """Q projection (row-blocks 0-2) for chunk tcn. Chunk 0 (prologue, no
    attention running yet) borrows the S-pair pool for double buffering."""
    nc = ctx["nc"]
    wqk_sb, bqk_sb = ctx["wqk_sb"], ctx["bqk_sb"]
    pool, tag = (ctx["pss"], "s") if tcn == 0 else (ctx["psqkv"], "qk")
    for rb in range(3):
        ps = pool.tile([128, NQ], F32, tag=tag)
        for cb in range(6):
            nc.tensor.matmul(
                ps, lhsT=wqk_sb[cb][:, rb * 128:(rb + 1) * 128],
                rhs=xc[:, cb, :], start=(cb == 0), stop=(cb == 5))
        with nc.allow_low_precision(reason="fp32r matmul operand"):
            nc.vector.tensor_scalar_add(QTc[rb][tcn], ps,
                                        bqk_sb[:, rb:rb + 1])


def emit_qkv_kv(ctx, tcn, KTc, Vc, xc):
    """K (row-blocks 3-5) + V projection for chunk tcn."""
    nc = ctx["nc"]
    wqk_sb, wv_sb = ctx["wqk_sb"], ctx["wv_sb"]
    bqk_sb, bv128 = ctx["bqk_sb"], ctx["bv128"]
    pool, tag = (ctx["pss"], "s") if tcn == 0 else (ctx["psqkv"], "qk")
    for rb in range(3, 6):
        ps = pool.tile([128, NQ], F32, tag=tag)
        for cb in range(6):
            nc.tensor.matmul(
                ps, lhsT=wqk_sb[cb][:, rb * 128:(rb + 1) * 128],
                rhs=xc[:, cb, :], start=(cb == 0), stop=(cb == 5))
        with nc.allow_low_precision(reason="fp32r matmul operand"):
            nc.vector.tensor_scalar_add(KTc[rb - 3][tcn], ps,
                                        bqk_sb[:, rb:rb + 1])
    # V: [t, r] layout, 4 t-subblocks; ones column fused for denominators
    Vc = Vc[tcn]
    nc.gpsimd.dma_start(
        out=Vc[:, :, :, HS],
        in_=ctx["ones"][:, 0:4 * HL].bitcast(F32R).rearrange(
            "p (a b) -> p a b", b=HL))
    for tb in range(4):
        psv = pool.tile([128, CL], F32, tag=tag)
        for cb in range(6):
            nc.tensor.matmul(
                psv, lhsT=xc[:, cb, tb * 128:(tb + 1) * 128],
                rhs=wv_sb[cb], start=(cb == 0), stop=(cb == 5))
        with nc.allow_low_precision(reason="fp32r matmul operand"):
            nc.vector.tensor_tensor(
                out=Vc[:, tb, :, 0:HS],
                in0=psv.rearrange("p (h d) -> p h d", d=HS),
                in1=bv128.rearrange("p (h d) -> p h d", d=HS), op=ADD)


def body(ctx, first):
    nc = ctx["nc"]
    qkvp = ctx["qkvp"]
    wp_sb, tri_sb = ctx["wp_sb"], ctx["tri_sb"]
    y_out = ctx["y_out"]
    ptp, otsbp, smallp, yevp = (ctx["ptp"], ctx["otsbp"], ctx["smallp"],
                                ctx["yevp"])
    pss, psot, psy = ctx["pss"], ctx["psot"], ctx["psy"]
    e2a_sb, e2b_sb = ctx["e2a_sb"], ctx["e2b_sb"]

    # per-chunk activation tiles: fine-grained deps let chunk J+1's QKV
    # projection overlap chunk J's attention
    QTc = [[qkvp.tile([128, NQ], F32R, tag=f"qt{rb}_{j}", name=f"qt{rb}_{j}")
            for j in range(NCH)] for rb in range(3)]
    KTc = [[qkvp.tile([128, NQ], F32R, tag=f"kt{rb}_{j}", name=f"kt{rb}_{j}")
            for j in range(NCH)] for rb in range(3)]
    Vc = [qkvp.tile([128, 4, HL, HS + 1], F32R, tag=f"v{j}", name=f"v{j}")
          for j in range(NCH)]

    xc = ctx["xc0"] if first else emit_xc_dma(ctx, 0)
    emit_qkv_q(ctx, 0, QTc, xc)
    emit_qkv_kv(ctx, 0, KTc, Vc, xc)

    for J in range(NCH):
        if J + 1 < NCH:
            # issue next chunk's x DMA early (SP/Pool are free)
            xcn = emit_xc_dma(ctx, J + 1)
        ots = [otsbp.tile([128, NQ], F32R, tag=f"ots{kb}", name=f"ots{kb}")
               for kb in range(3)]
        for kb in range(3):
            ot = [psot.tile([HS + 1, NQ], F32, tag="ot", name=f"ot{_h}")
                  for _h in range(2)]
            # software pipeline: issue S(t)/exp(t) for both heads, then the
            # deferred O(t-1) pair, so the PE works ahead of ACT's exp.
            pend = []
            for t in range(J * 4 + 4):
                tc_i, tb = divmod(t, 4)
                diag = t - J * 4             # >= 0 on diagonal tiles
                W = NQ if diag < 0 else NQ - 128 * diag
                new = []
                # both heads' S^T tiles go into one 2-bank PSUM tile so a
                # single ACT instruction computes exp for the pair
                sps = pss.tile([128, 2, NQ], F32, tag="s")
                pt = ptp.tile([128, 2, NQ], F32R, tag="pt")
                for hh in range(2):
                    po = hh * HS
                    kt = KTc[kb][tc_i][po:po + HS, tb * 128:(tb + 1) * 128]
                    nc.tensor.matmul(
                        sps[:, hh, 0:W], lhsT=kt,
                        rhs=(QTc[kb][J][po:po + HS, :] if diag < 0 else
                             QTc[kb][J][po:po + HS, 128 * diag:NQ]),
                        start=True, stop=True)
                nc.scalar.activation(pt[:, :, 0:W], sps[:, :, 0:W], EXP,
                                     scale=0.125)
                if diag >= 0:
                    nc.vector.tensor_mul(pt[:, 0, 0:128], pt[:, 0, 0:128],
                                         tri_sb)
                    nc.vector.tensor_mul(pt[:, 1, 0:128], pt[:, 1, 0:128],
                                         tri_sb)
                for hh in range(2):
                    dst = ot[hh] if diag < 0 else ot[hh][:, 128 * diag:NQ]
                    new.append(dict(out=dst, lhsT=Vc[tc_i][:, tb, 2 * kb + hh, :],
                                    rhs=pt[:, hh, 0:W], start=(t == 0),
                                    stop=False))
                for o in pend:
                    nc.tensor.matmul(o.pop("out"), **o)
                pend = new
            for o in pend:
                o["stop"] = True
                nc.tensor.matmul(o.pop("out"), **o)
            # normalize the pair: reciprocal rows -> rank-1 broadcast -> mul
            reca = smallp.tile([1, NQ], F32R, tag="reca")
            recb = smallp.tile([1, NQ], F32R, tag="recb")
            with nc.allow_low_precision(reason="fp32r matmul operand"):
                nc.vector.reciprocal(reca, ot[0][HS:HS + 1, :])
                nc.vector.reciprocal(recb, ot[1][HS:HS + 1, :])
            bc = psy.tile([128, NQ], F32, tag="yps")
            nc.tensor.matmul(bc, lhsT=e2a_sb, rhs=reca, start=True, stop=False)
            nc.tensor.matmul(bc, lhsT=e2b_sb, rhs=recb, start=False, stop=True)
            bcs = smallp.tile([128, NQ], F32, tag="bcs")
            nc.vector.tensor_copy(bcs, bc)
            nc.vector.tensor_mul(ots[kb][0:HS, :], ot[0][0:HS, :],
                                 bcs[0:HS, :])
            nc.vector.tensor_mul(ots[kb][HS:128, :], ot[1][0:HS, :],
                                 bcs[HS:128, :])
        # c_proj for this q-block; the last chunk borrows the (by then idle)
        # S-pair pool so its accumulation groups double-buffer in the tail
        ypool, ytag = (pss, "s") if J == NCH - 1 else (psy, "yps")
        for i in range(4):
            ti = J * 4 + i
            yt = yevp.tile([128, C], F32, tag="yt")
            for half in range(2):
                yps = ypool.tile([128, CL], F32, tag=ytag)
                for cb in range(3):
                    nc.tensor.matmul(
                        yps, lhsT=ots[cb][:, i * 128:(i + 1) * 128],
                        rhs=wp_sb[cb][:, half * CL:(half + 1) * CL],
                        start=(cb == 0), stop=(cb == 2))
                nc.vector.tensor_copy(yt[:, half * CL:(half + 1) * CL], yps)
            nc.sync.dma_start(out=y_out[ti * 128:(ti + 1) * 128, :], in_=yt)
        # project the next chunk, emitted last (= lowest priority): the
        # scheduler pulls these matmuls into PE stalls of the ACT-bound
        # attention loops above
        if J + 1 < NCH:
            emit_qkv_q(ctx, J + 1, QTc, xcn)
            emit_qkv_kv(ctx, J + 1, KTc, Vc, xcn)


def make_in_maps(x, w_attn, b_attn, w_proj):
    x = np.asarray(x, dtype=np.float32)
    w_attn = np.asarray(w_attn, dtype=np.float32)
    b_attn = np.asarray(b_attn, dtype=np.float32)
    w_proj = np.asarray(w_proj, dtype=np.float32)
    # valid iff kv <= q with kv on partitions (rows), q on free dim (cols)
    tri = np.triu(np.ones((128, 128), dtype=np.float32))
    e2 = np.zeros((2, 128), dtype=np.float32)
    e2[0, 0:HS] = 1.0
    e2[1, HS:128] = 1.0
    in_maps = []
    for core in range(NCORES):
        b, g = divmod(core, 2)
        sl = slice(g * CL, (g + 1) * CL)
        wq, wk, wv = (w_attn[i * C:(i + 1) * C][sl] for i in range(3))
        bq, bk, bv_ = (b_attn[i * C:(i + 1) * C][sl] for i in range(3))
        bqkc = np.concatenate([bq, bk]).reshape(6, 128).T.copy()
        in_maps.append({
            "xT": np.ascontiguousarray(x[b].T),
            "wqkT": np.ascontiguousarray(np.concatenate([wq, wk], 0).T),
            "wvT": np.ascontiguousarray(wv.T),
            "wpT": np.ascontiguousarray(w_proj[:, sl].T),
            "bqkc": bqkc,
            "bv": bv_[None, :].copy(),
            "tri": tri,
            "ones": np.ones((128, 128), dtype=np.float32),
            "e2": e2,
        })
    return in_maps


def assemble(results, b_proj):
    out = np.empty((B, T, C), dtype=np.float32)
    for b in range(B):
        out[b] = results[2 * b]["y_out"] + results[2 * b + 1]["y_out"]
    out += np.asarray(b_proj, dtype=np.float32)[None, None, :]
    return out


_CACHE = {}


def _get_nc():
    if "nc" not in _CACHE:
        _CACHE["nc"] = build_bass()
    return _CACHE["nc"]


def kernel(x, w_attn, b_attn, w_proj, b_proj):
    in_maps = make_in_maps(x, w_attn, b_attn, w_proj)
    res = run_bass_kernel_spmd(_get_nc(), in_maps, list(range(NCORES)))
    return assemble(res.results, b_proj)
